# revision 3
# baseline (speedup 1.0000x reference)
"""AMNet GNN message passing on 8 TRN2 NeuronCores.

Strategy (v3)
-------------
The graph propagation ``prop(v) = scatter_add(ew * v[src]) + 0.5*v`` is run as
5 dense GEMM hops against a host-materialized normalized operator, with the
(all-gathered) node features V as the *stationary* PE weights (fp8 DoubleRow)
and the resident P.T tiles streaming as the moving operand, producing hop
output feature-major ([HID, nodes_local]).

v3 changes vs v2 (trace-driven):
  - P.T (13.1MB/core) is host-linearized to [128, KT*SH] (contiguous per
    partition) and bulk-loaded as 40 per-kpair DMAs on the gpsimd SWDGE
    queue in KORDER. In v2 it sat at the head of the sync HWDGE queue and
    stalled the first exchange (and hence hop 1) until ~110us.
  - DMA queue split: exchange transposes + bounce writes + final output
    writes on the sync queue; reloads + xt/rhs loads on the scalar queue.
    Reloads are issued immediately after their AllGather (they can no
    longer head-of-line block the transposes).
  - Node segments rebalanced A=768 / B=512 (was 512/768): phase 1 is now
    the longer phase, giving the B AllGather ~17us of phase-1 contraction
    (KPA = 24 pairs) to land before the KPB tiles are consumed.
  - hf accumulation (hf_f += alpha * S_h) split per segment and spread
    across DVE and GpSimd so the next hop's PSUM drains don't queue
    behind it on the DVE FIFO.
  - Attention runs segment-major (full logits/softmax/res chain on
    segment A first) so the res-A AllGather fires ~25us earlier and the
    final distributed GEMM starts sooner.

Nodes are padded 10000 -> 10240; every core owns 1280 rows; SPMD on 8 cores.
"""

import sys

for _p in ("/opt/trn_rl_repo", "/root/.axon_site/_ro/trn_rl_repo"):
    if _p not in sys.path:
        sys.path.append(_p)

import math

import ml_dtypes
import numpy as np

import concourse.bass as bass
import concourse.mybir as mybir
import concourse.tile as tile
from concourse import bacc
from concourse.bass_utils import run_bass_kernel_spmd

N, E, IN, HID, KDEG, FN = 10000, 640000, 512, 256, 5, 5
NCORES = 8
NPAD = 10240                 # padded node count (8 * 1280)
SH = NPAD // NCORES          # 1280 nodes per core
MT = SH // 128               # 10 node tiles per core
KT = NPAD // 128             # 80 contraction tiles (all nodes)
KP = KT // 2                 # 40 DoubleRow contraction pairs
HT = HID // 128              # 2 feature tiles
INT = IN // 128              # 4 input-feature tiles

BF16 = mybir.dt.bfloat16
F32 = mybir.dt.float32
FP8 = mybir.dt.float8e4

S_P = 64.0                   # P fp8 scale (entries ~0.008 -> ~0.5)
S_V = 8.0                    # V fp8 scale
DESCALE = 2.0 / (S_P * S_V)  # hop drain: S_new = psum*DESCALE + S_old

# node segments (local): A = tiles 0..5 (768 nodes), B = tiles 6..9 (512)
NA, NB = 768, SH - 768
NTA, NTB = NA // 128, NB // 128          # 6 / 4 tiles
KP_R = MT // 2                           # 5 kpairs per rank
KPA_R, KPB_R = NTA // 2, NTB // 2        # 3 / 2 kpairs per rank per segment
# contraction k-pair order: pairs covering A-tiles of every rank first, so
# the next hop can start on AllGather-A data while segment B still exchanges
KPA = [r * KP_R + i for r in range(NCORES) for i in range(KPA_R)]
KPB = [r * KP_R + KPA_R + i for r in range(NCORES) for i in range(KPB_R)]
KORDER = KPA + KPB

# MLP strips: segment A = strips 0+1, segment B = strip 2
NSTRIP = [(0, 512), (512, 256), (768, 512)]
# per-segment strips for attention (psum <= 512 wide)
SEGSTRIP = {"a": [(0, 512), (512, 256)], "b": [(768, 512)]}
SEGOF = {"a": (0, NA), "b": (NA, NB)}


def _bern_coeff(K):
    out = np.zeros((K + 1, K + 1), dtype=np.float64)
    P = np.polynomial.polynomial.Polynomial
    for i in range(K + 1):
        c = np.zeros(i + 1)
        c[i] = math.comb(K, i)
        p = P(c) * (P([1.0, -1.0]) ** (K - i))
        out[i, : len(p.coef)] = p.coef
    return out


def _build_graph():
    nc = bacc.Bacc("TRN2", target_bir_lowering=False, debug=False, num_devices=NCORES)

    # ---- per-core parameters ----
    xt = nc.declare_dram_parameter("xt", [IN, SH], BF16, isOutput=False)
    ptt = nc.declare_dram_parameter("ptt", [128, KT * SH], FP8, isOutput=False)
    w1 = nc.declare_dram_parameter("w1", [IN, HID], BF16, isOutput=False)
    w2 = nc.declare_dram_parameter("w2", [HID, HID], BF16, isOutput=False)
    wf = nc.declare_dram_parameter("wf", [HID, HID], BF16, isOutput=False)
    wx = nc.declare_dram_parameter("wx", [HID, HID], BF16, isOutput=False)
    b1 = nc.declare_dram_parameter("b1", [128, HT], F32, isOutput=False)
    b2 = nc.declare_dram_parameter("b2", [128, HT], F32, isOutput=False)
    wfb = nc.declare_dram_parameter("wfb", [128, HT], F32, isOutput=False)
    wxb = nc.declare_dram_parameter("wxb", [128, HT], F32, isOutput=False)
    alpha = nc.declare_dram_parameter("alpha", [128, FN * (KDEG + 1)], F32,
                                      isOutput=False)
    colsel = nc.declare_dram_parameter("colsel", [128, FN * 128], BF16,
                                       isOutput=False)
    rowsel = nc.declare_dram_parameter("rowsel", [128, FN * 128], BF16,
                                       isOutput=False)
    allones = nc.declare_dram_parameter("allones", [128, 128], BF16,
                                        isOutput=False)
    out = nc.declare_dram_parameter("out", [SH, NPAD], BF16, isOutput=True)

    rg = [list(range(NCORES))]

    with tile.TileContext(nc) as tc:
        with tc.tile_pool(name="dram", bufs=1, space="DRAM") as dram, \
             tc.tile_pool(name="consts", bufs=1) as consts, \
             tc.tile_pool(name="persist", bufs=1) as persist:

            # ---- DRAM internals for collectives ----
            # hop exchanges e=0..4 (e=0 is the MLP output h); node-major fp8
            # v8 layout per core: [128, hh, nt*128] with node = t*128 + p
            bounce_a = [dram.tile([128, HT, NA], FP8, name=f"bnc_a{e}")
                        for e in range(KDEG)]
            bounce_b = [dram.tile([128, HT, NB], FP8, name=f"bnc_b{e}")
                        for e in range(KDEG)]
            vfull_a = [dram.tile([NCORES, 128, HT, NA], FP8, addr_space="Shared",
                                 name=f"vf_a{e}") for e in range(KDEG)]
            vfull_b = [dram.tile([NCORES, 128, HT, NB], FP8, addr_space="Shared",
                                 name=f"vf_b{e}") for e in range(KDEG)]
            # res exchange (bf16, feature-major: res.T[hid, n])
            bounce_ra = dram.tile([128, HT, NA], BF16)
            bounce_rb = dram.tile([128, HT, NB], BF16)
            vfull_ra = dram.tile([NCORES, 128, HT, NA], BF16, addr_space="Shared")
            vfull_rb = dram.tile([NCORES, 128, HT, NB], BF16, addr_space="Shared")

            # ---- constants to SBUF ----
            w1_sb = consts.tile([128, INT, HID], BF16)
            nc.sync.dma_start(w1_sb[:], w1[:, :].rearrange("(kt p) n -> p kt n", p=128))
            w2_sb = consts.tile([128, HT, HID], BF16)
            nc.sync.dma_start(w2_sb[:], w2[:, :].rearrange("(kt p) n -> p kt n", p=128))
            wf_sb = consts.tile([128, HT, HID], BF16)
            nc.sync.dma_start(wf_sb[:], wf[:, :].rearrange("(kt p) n -> p kt n", p=128))
            wx_sb = consts.tile([128, HT, HID], BF16)
            nc.sync.dma_start(wx_sb[:], wx[:, :].rearrange("(kt p) n -> p kt n", p=128))
            b1_sb = consts.tile([128, HT], F32)
            nc.sync.dma_start(b1_sb[:], b1[:, :])
            b2_sb = consts.tile([128, HT], F32)
            nc.sync.dma_start(b2_sb[:], b2[:, :])
            wfb_sb = consts.tile([128, HT], F32)
            nc.sync.dma_start(wfb_sb[:], wfb[:, :])
            wxb_sb = consts.tile([128, HT], F32)
            nc.sync.dma_start(wxb_sb[:], wxb[:, :])
            alpha_sb = consts.tile([128, FN * (KDEG + 1)], F32)
            nc.sync.dma_start(alpha_sb[:], alpha[:, :])
            colsel_sb = consts.tile([128, FN * 128], BF16)
            nc.sync.dma_start(colsel_sb[:], colsel[:, :])
            rowsel_sb = consts.tile([128, FN * 128], BF16)
            nc.sync.dma_start(rowsel_sb[:], rowsel[:, :])
            ones_sb = consts.tile([128, 128], BF16)
            nc.sync.dma_start(ones_sb[:], allones[:, :])

            # ---- persistent activations ----
            s_sb = persist.tile([128, HT, SH], BF16)      # S_h = 2^h B_h (feat-major)
            xp_sb = persist.tile([128, HT, SH], BF16)     # x_proj.T
            res_sb = persist.tile([128, HT, SH], BF16)    # res.T
            hf_sb = persist.tile([128, FN, HT, SH], BF16)  # h_filters.T accumulators

            def a_of(f, h):
                return alpha_sb[:, f * (KDEG + 1) + h: f * (KDEG + 1) + h + 1]

            with tc.tile_pool(name="hop", bufs=1) as hop, \
                 tc.tile_pool(name="hop_ps", bufs=1, space="PSUM") as hop_ps, \
                 tc.tile_pool(name="mlp_ps", bufs=2, space="PSUM") as mlp_ps:

                pt_sb = hop.tile([128, KT, SH], FP8)
                # resident P.T bulk load on the gpsimd SWDGE queue in KORDER:
                # contiguous 2*SH bytes per partition per kpair, and it never
                # blocks the latency-critical sync/scalar HWDGE queues
                for kp in KORDER:
                    nc.gpsimd.dma_start(
                        pt_sb[:, 2 * kp:2 * kp + 2, :],
                        ptt[:, 2 * kp * SH:(2 * kp + 2) * SH].rearrange(
                            "p (t n) -> p t n", n=SH),
                    )

                vt_ctx = tc.tile_pool(name="vt", bufs=1)
                vt_pool = vt_ctx.__enter__()

                # ---- exchange helpers ----
                def exchange(e, seg):
                    """Transpose S segment to node-major, cast fp8, AllGather."""
                    n0, nw = SEGOF[seg]
                    vt = vt_pool.tile([128, HT, SH], BF16, tag="vt",
                                      name=f"vt{e}{seg}")
                    v8 = vt_pool.tile([128, HT, SH], FP8, tag="v8",
                                      name=f"v8{e}{seg}")
                    bnc = (bounce_a[e] if seg == "a" else bounce_b[e])
                    vf = (vfull_a[e] if seg == "a" else vfull_b[e])
                    for hh in range(HT):
                        # [128 hid, nw nodes] -> [128 node-part, nw/128, 128 hid]
                        nc.sync.dma_start_transpose(
                            vt[:, hh, n0:n0 + nw].rearrange(
                                "p (t q) -> p t q", q=128),
                            s_sb[:, hh, n0:n0 + nw],
                        )
                        nc.scalar.activation(
                            v8[:, hh, n0:n0 + nw], vt[:, hh, n0:n0 + nw],
                            mybir.ActivationFunctionType.Copy, scale=S_V,
                        )
                        nc.sync.dma_start(bnc[:, hh, :], v8[:, hh, n0:n0 + nw])
                    nc.gpsimd.collective_compute(
                        "AllGather", mybir.AluOpType.bypass,
                        ins=[bnc.opt()], outs=[vf.opt()], replica_groups=rg,
                    )
                    return vf

                def reload(v_sb, vf, seg):
                    # scalar HWDGE queue: waits on the AllGather semaphore
                    # without blocking the sync-queue transposes/bounces
                    ntt = NTA if seg == "a" else NTB
                    for r in range(NCORES):
                        nc.scalar.dma_start(
                            v_sb[:, :, r * ntt:(r + 1) * ntt, :]
                            .rearrange("p h t q -> p h (t q)"),
                            vf[r],
                        )

                # ======== MLP: S_0 = h.T = (relu(x@W1+b1)@W2+b2).T ========
                # strip-ordered so the A-segment exchange fires ASAP
                with tc.tile_pool(name="mlp", bufs=1) as mlp:
                    xt_sb = mlp.tile([128, INT, SH], BF16)
                    xt_r = xt[:, :].rearrange("(kt p) n -> p kt n", p=128)
                    for kt in range(INT):
                        nc.scalar.dma_start(xt_sb[:, kt, :], xt_r[:, kt, :])

                    h1_sb = mlp.tile([128, HT, SH], BF16)
                    for si, (n0, nw) in enumerate(NSTRIP):
                        for hh in range(HT):
                            ps = mlp_ps.tile([128, 512], F32, name="mlp_ps")
                            for k in range(INT):
                                nc.tensor.matmul(
                                    ps[:, :nw],
                                    w1_sb[:, k, hh * 128:(hh + 1) * 128],
                                    xt_sb[:, k, n0:n0 + nw],
                                    start=(k == 0), stop=(k == INT - 1),
                                )
                            nc.scalar.activation(
                                h1_sb[:, hh, n0:n0 + nw], ps[:, :nw],
                                mybir.ActivationFunctionType.Relu,
                                bias=b1_sb[:, hh:hh + 1],
                            )
                        for hh in range(HT):
                            ps = mlp_ps.tile([128, 512], F32, name="mlp_ps2",
                                             tag="mlp_ps")
                            for k in range(HT):
                                nc.tensor.matmul(
                                    ps[:, :nw],
                                    w2_sb[:, k, hh * 128:(hh + 1) * 128],
                                    h1_sb[:, k, n0:n0 + nw],
                                    start=(k == 0), stop=(k == HT - 1),
                                )
                            nc.scalar.activation(
                                s_sb[:, hh, n0:n0 + nw], ps[:, :nw],
                                mybir.ActivationFunctionType.Identity,
                                bias=b2_sb[:, hh:hh + 1],
                            )
                        if si == 1:
                            vf0a = exchange(0, "a")
                    vf0b = exchange(0, "b")

                    # x_proj.T = tanh(wx.T @ h.T + wxb) - fills the AG wait
                    for hh in range(HT):
                        for n0, nw in NSTRIP:
                            ps = mlp_ps.tile([128, 512], F32, name="xp_ps",
                                             tag="mlp_ps")
                            for k in range(HT):
                                nc.tensor.matmul(
                                    ps[:, :nw],
                                    wx_sb[:, k, hh * 128:(hh + 1) * 128],
                                    s_sb[:, k, n0:n0 + nw],
                                    start=(k == 0), stop=(k == HT - 1),
                                )
                            nc.scalar.activation(
                                xp_sb[:, hh, n0:n0 + nw], ps[:, :nw],
                                mybir.ActivationFunctionType.Tanh,
                                bias=wxb_sb[:, hh:hh + 1],
                            )

                # hf init: hf_f = alpha'[f,0] * S_0
                for f in range(FN):
                    for hh in range(HT):
                        nc.scalar.activation(
                            hf_sb[:, f, hh, :], s_sb[:, hh, :],
                            mybir.ActivationFunctionType.Copy,
                            scale=a_of(f, 0),
                        )

                # v weights pool opens after the MLP pool frees its space
                # (split into A/B tiles so segment-B reloads don't serialize
                # the phase-1/2 matmuls that only read segment-A weights)
                vsb_ctx = tc.tile_pool(name="vsb", bufs=2)
                vsb_pool = vsb_ctx.__enter__()

                def valloc(h):
                    va = vsb_pool.tile([128, HT, NCORES * NTA, 128], FP8,
                                       tag="v_a", name=f"v_a{h}")
                    vb = vsb_pool.tile([128, HT, NCORES * NTB, 128], FP8,
                                       tag="v_b", name=f"v_b{h}")
                    return va, vb

                def vslice(va, vb, i, hh):
                    if i < len(KPA):
                        return va[:, hh, 2 * i:2 * i + 2, :]
                    j = i - len(KPA)
                    return vb[:, hh, 2 * j:2 * j + 2, :]

                v_cur = valloc(0)
                reload(v_cur[0], vf0a, "a")
                reload(v_cur[1], vf0b, "b")

                # hf updates per segment (issued while the other segment's
                # matmuls run) so the drains never queue behind a full-width
                # update burst; scalar-AP STT is DVE-only (Pool lacks the op)
                def hf_update(h, seg):
                    n0, nw = SEGOF[seg]
                    for f in range(FN):
                        eng = nc.vector
                        for hh in range(HT):
                            eng.scalar_tensor_tensor(
                                out=hf_sb[:, f, hh, n0:n0 + nw],
                                in0=s_sb[:, hh, n0:n0 + nw],
                                scalar=a_of(f, h),
                                in1=hf_sb[:, f, hh, n0:n0 + nw],
                                op0=mybir.AluOpType.mult,
                                op1=mybir.AluOpType.add,
                            )

                # ======== 5 propagation hops (feature-major output) ========
                # phase 1: segment A outputs (cols 0..767, psum 512+256)
                # phase 2: segment B outputs (cols 768..1279, psum 512)
                psA1 = [hop_ps.tile([128, 512], F32, name=f"psA1{hh}")
                        for hh in range(HT)]
                psA2 = [hop_ps.tile([128, 256], F32, name=f"psA2{hh}")
                        for hh in range(HT)]
                psB = [hop_ps.tile([128, 512], F32, name=f"psB{hh}")
                       for hh in range(HT)]

                for h in range(1, KDEG + 1):
                    va, vb = v_cur
                    # phase 1: output nodes 0..767 (segment A)
                    for i, kp in enumerate(KORDER):
                        st, sp = (i == 0), (i == KP - 1)
                        for hh in range(HT):
                            nc.tensor.matmul(
                                psA1[hh][:, :],
                                vslice(va, vb, i, hh),
                                pt_sb[:, 2 * kp:2 * kp + 2, 0:512],
                                start=st, stop=sp,
                                perf_mode=mybir.MatmulPerfMode.DoubleRow,
                            )
                            nc.tensor.matmul(
                                psA2[hh][:, :],
                                vslice(va, vb, i, hh),
                                pt_sb[:, 2 * kp:2 * kp + 2, 512:NA],
                                start=st, stop=sp,
                                perf_mode=mybir.MatmulPerfMode.DoubleRow,
                            )
                    # drain A (fused off-diag descale + self-loop add, bf16)
                    for hh in range(HT):
                        nc.vector.scalar_tensor_tensor(
                            out=s_sb[:, hh, 0:512], in0=psA1[hh][:, :],
                            scalar=DESCALE, in1=s_sb[:, hh, 0:512],
                            op0=mybir.AluOpType.mult, op1=mybir.AluOpType.add,
                        )
                        nc.vector.scalar_tensor_tensor(
                            out=s_sb[:, hh, 512:NA], in0=psA2[hh][:, :],
                            scalar=DESCALE, in1=s_sb[:, hh, 512:NA],
                            op0=mybir.AluOpType.mult, op1=mybir.AluOpType.add,
                        )
                    if h < KDEG:
                        vfa = exchange(h, "a")
                        v_cur = valloc(h)
                        reload(v_cur[0], vfa, "a")
                    hf_update(h, "a")
                    # phase 2: output nodes 768..1279 (segment B)
                    for i, kp in enumerate(KORDER):
                        st, sp = (i == 0), (i == KP - 1)
                        for hh in range(HT):
                            nc.tensor.matmul(
                                psB[hh][:, :],
                                vslice(va, vb, i, hh),
                                pt_sb[:, 2 * kp:2 * kp + 2, NA:SH],
                                start=st, stop=sp,
                                perf_mode=mybir.MatmulPerfMode.DoubleRow,
                            )
                    for hh in range(HT):
                        nc.vector.scalar_tensor_tensor(
                            out=s_sb[:, hh, NA:SH], in0=psB[hh][:, :],
                            scalar=DESCALE, in1=s_sb[:, hh, NA:SH],
                            op0=mybir.AluOpType.mult, op1=mybir.AluOpType.add,
                        )
                    if h < KDEG:
                        vfb = exchange(h, "b")
                        reload(v_cur[1], vfb, "b")
                    hf_update(h, "b")

                vsb_ctx.__exit__(None, None, None)
                vt_ctx.__exit__(None, None, None)

            # ======== attention fusion (feature-major, segment-major) ========
            with tc.tile_pool(name="attn", bufs=1) as attn, \
                 tc.tile_pool(name="attn_ps", bufs=2, space="PSUM") as attn_ps, \
                 tc.tile_pool(name="sc_ps", bufs=3, space="PSUM") as sc_ps, \
                 tc.tile_pool(name="lg_ps", bufs=1, space="PSUM") as lg_ps, \
                 tc.tile_pool(name="tmp2", bufs=3) as tmp2:

                hfp_sb = attn.tile([128, FN, HT, SH], BF16)
                expT = attn.tile([FN, SH], BF16)
                rinv = attn.tile([128, SH], F32)
                score_sb = attn.tile([128, FN, SH], BF16)

                for seg in ("a", "b"):
                    strips = SEGSTRIP[seg]
                    sn0, snw = SEGOF[seg]
                    # hfp_f.T = tanh(wf.T @ hf_f.T + wfb)
                    for f in range(FN):
                        for hh in range(HT):
                            for n0, nw in strips:
                                ps = attn_ps.tile([128, 512], F32,
                                                  name="hfp_ps", tag="hfp_ps")
                                for k in range(HT):
                                    nc.tensor.matmul(
                                        ps[:, :nw],
                                        wf_sb[:, k, hh * 128:(hh + 1) * 128],
                                        hf_sb[:, f, k, n0:n0 + nw],
                                        start=(k == 0), stop=(k == HT - 1),
                                    )
                                nc.scalar.activation(
                                    hfp_sb[:, f, hh, n0:n0 + nw], ps[:, :nw],
                                    mybir.ActivationFunctionType.Tanh,
                                    bias=wfb_sb[:, hh:hh + 1],
                                )

                    # logits: logit[f, n] = sum_hid hfp_f.T * xp.T
                    # accumulated across (f, hh) into one psum via col-select
                    psL = [lg_ps.tile([128, nw], F32, name=f"psL{seg}{si}",
                                      tag=f"psL{si}")
                           for si, (n0, nw) in enumerate(strips)]
                    for f in range(FN):
                        for hh in range(HT):
                            tmp = tmp2.tile([128, SH], BF16, tag="lg_tmp")
                            nc.vector.tensor_mul(
                                out=tmp[:, sn0:sn0 + snw],
                                in0=hfp_sb[:, f, hh, sn0:sn0 + snw],
                                in1=xp_sb[:, hh, sn0:sn0 + snw],
                            )
                            for si, (n0, nw) in enumerate(strips):
                                nc.tensor.matmul(
                                    psL[si][:, :],
                                    colsel_sb[:, f * 128:(f + 1) * 128],
                                    tmp[:, n0:n0 + nw],
                                    start=(f == 0 and hh == 0),
                                    stop=(f == FN - 1 and hh == HT - 1),
                                )
                    # exp (logits are tiny dot products; no max-sub needed)
                    for si, (n0, nw) in enumerate(strips):
                        nc.scalar.activation(
                            expT[:, n0:n0 + nw], psL[si][0:FN, :],
                            mybir.ActivationFunctionType.Exp,
                        )
                    # broadcast sum over filters + reciprocal
                    for si, (n0, nw) in enumerate(strips):
                        psS = sc_ps.tile([128, 512], F32, name="psS", tag="sc")
                        nc.tensor.matmul(
                            psS[:, :nw], ones_sb[0:FN, :], expT[:, n0:n0 + nw],
                            start=True, stop=True,
                        )
                        nc.vector.reciprocal(rinv[:, n0:n0 + nw], psS[:, :nw])
                    # score_f broadcast to 128 partitions: (1 x exp_f) * rinv
                    for f in range(FN):
                        for si, (n0, nw) in enumerate(strips):
                            psb = sc_ps.tile([128, 512], F32, name="psb",
                                             tag="sc")
                            nc.tensor.matmul(
                                psb[:, :nw],
                                rowsel_sb[0:FN, f * 128:(f + 1) * 128],
                                expT[0:FN, n0:n0 + nw],
                                start=True, stop=True,
                            )
                            nc.vector.tensor_mul(
                                out=score_sb[:, f, n0:n0 + nw],
                                in0=psb[:, :nw],
                                in1=rinv[:, n0:n0 + nw],
                            )

                    # res.T = sum_f score_f * hf_f.T  (DVE + GpSimd split)
                    for hh in range(HT):
                        nc.vector.tensor_mul(
                            out=res_sb[:, hh, sn0:sn0 + snw],
                            in0=score_sb[:, 0, sn0:sn0 + snw],
                            in1=hf_sb[:, 0, hh, sn0:sn0 + snw],
                        )
                        tmpg = tmp2.tile([128, HT, SH], BF16, tag="res_tmpg")
                        nc.gpsimd.tensor_mul(
                            out=tmpg[:, hh, sn0:sn0 + snw],
                            in0=score_sb[:, 1, sn0:sn0 + snw],
                            in1=hf_sb[:, 1, hh, sn0:sn0 + snw],
                        )
                        nc.vector.tensor_add(
                            out=res_sb[:, hh, sn0:sn0 + snw],
                            in0=res_sb[:, hh, sn0:sn0 + snw],
                            in1=tmpg[:, hh, sn0:sn0 + snw],
                        )
                        for f in range(2, FN):
                            eng = nc.gpsimd if f % 2 == 0 else nc.vector
                            tmp = tmp2.tile([128, HT, SH], BF16, tag="res_tmp")
                            eng.tensor_mul(
                                out=tmp[:, hh, sn0:sn0 + snw],
                                in0=score_sb[:, f, sn0:sn0 + snw],
                                in1=hf_sb[:, f, hh, sn0:sn0 + snw],
                            )
                            nc.vector.tensor_add(
                                out=res_sb[:, hh, sn0:sn0 + snw],
                                in0=res_sb[:, hh, sn0:sn0 + snw],
                                in1=tmp[:, hh, sn0:sn0 + snw],
                            )
                    bnc = bounce_ra if seg == "a" else bounce_rb
                    vf = vfull_ra if seg == "a" else vfull_rb
                    nc.sync.dma_start(bnc[:, :, :], res_sb[:, :, sn0:sn0 + snw])
                    nc.gpsimd.collective_compute(
                        "AllGather", mybir.AluOpType.bypass,
                        ins=[bnc.opt()], outs=[vf.opt()], replica_groups=rg,
                    )

            # ======== final distributed GEMM: out_r = res_r @ res.T (bf16) ====
            with tc.tile_pool(name="fin", bufs=1) as fin, \
                 tc.tile_pool(name="stage", bufs=3) as stage_pool, \
                 tc.tile_pool(name="fin_ps", bufs=6, space="PSUM") as fin_ps:
                rhs_sb = fin.tile([128, HT, NPAD], BF16)
                for r in range(NCORES):
                    nc.scalar.dma_start(
                        rhs_sb[:, :, r * SH:r * SH + NA], vfull_ra[r]
                    )
                for r in range(NCORES):
                    nc.scalar.dma_start(
                        rhs_sb[:, :, r * SH + NA:(r + 1) * SH], vfull_rb[r]
                    )
                out_r = out[:, :].rearrange("(t p) f -> p t f", p=128)
                CHUNKS_A = [(r * SH + c0, cw) for r in range(NCORES)
                            for c0, cw in ((0, 512), (512, 256))]
                CHUNKS_B = [(r * SH + NA, 512) for r in range(NCORES)]
                # all A-halves first: they only need the res-A AllGather, and
                # their output writes overlap the whole B sweep
                for half, chunks in (("a", CHUNKS_A), ("b", CHUNKS_B)):
                    for m in range(MT):
                        stg = stage_pool.tile([128, NPAD], BF16, tag="stage")
                        stg_r = stg.rearrange("p (r q) -> p r q", q=SH)
                        dst_r = out_r[:, m, :].rearrange("p (r q) -> p r q",
                                                         q=SH)
                        for ci, (c0, cw) in enumerate(chunks):
                            ps = fin_ps.tile([128, 512], F32, name="fin_ps")
                            for k in range(HT):
                                nc.tensor.matmul(
                                    ps[:, :cw],
                                    res_sb[:, k, m * 128:(m + 1) * 128],
                                    rhs_sb[:, k, c0:c0 + cw],
                                    start=(k == 0), stop=(k == HT - 1),
                                )
                            if ci % 2 == 1:
                                nc.scalar.copy(stg[:, c0:c0 + cw], ps[:, :cw])
                            else:
                                nc.vector.tensor_copy(stg[:, c0:c0 + cw],
                                                      ps[:, :cw])
                        if half == "a":
                            nc.sync.dma_start(dst_r[:, :, :NA],
                                              stg_r[:, :, :NA])
                        else:
                            nc.sync.dma_start(dst_r[:, :, NA:],
                                              stg_r[:, :, NA:])
    nc.compile()
    return nc


_GRAPH_CACHE = {}


def _get_graph():
    if "nc" not in _GRAPH_CACHE:
        _GRAPH_CACHE["nc"] = _build_graph()
    return _GRAPH_CACHE["nc"]


def prepare_in_maps(x, edge_index, lin1_w, lin1_b, lin2_w, lin2_b, filt_w,
                    wf_w, wf_b, wx_w, wx_b):
    x = np.asarray(x, np.float32)
    edge_index = np.asarray(edge_index)
    src = edge_index[0].astype(np.int64)
    dst = edge_index[1].astype(np.int64)

    # ---- host prep: dense normalized operator, OFF-DIAGONAL only, x S_P ----
    deg = np.zeros(N, np.float32)
    np.add.at(deg, src, np.float32(1.0))
    dinv = np.where(deg > 0, 1.0 / np.sqrt(deg), 0.0).astype(np.float32)
    ew = (-(dinv[src] * dinv[dst]) * (0.5 * S_P)).astype(np.float32)
    W = np.zeros((NPAD, NPAD), np.float32)
    np.add.at(W, (dst, src), ew)
    f8 = ml_dtypes.float8_e4m3
    W8 = W.astype(f8)
    del W
    W8T = np.ascontiguousarray(W8.T)
    del W8

    coeff = _bern_coeff(KDEG).astype(np.float32)
    fw = 1.0 / (1.0 + np.exp(-np.asarray(filt_w, np.float32)))
    al = (fw @ coeff).astype(np.float32)                  # [FN, KDEG+1]
    al = al * (0.5 ** np.arange(KDEG + 1))[None, :]       # absorb S_h = 2^h B_h
    alpha_bc = np.repeat(al.reshape(1, -1), 128, 0).astype(np.float32)

    xpad = np.zeros((NPAD, IN), np.float32)
    xpad[:N] = x

    bf = ml_dtypes.bfloat16
    w1_b = np.ascontiguousarray(np.asarray(lin1_w, np.float32)).astype(bf)
    w2_b = np.ascontiguousarray(np.asarray(lin2_w, np.float32)).astype(bf)
    wf_bm = np.ascontiguousarray(np.asarray(wf_w, np.float32)).astype(bf)
    wx_bm = np.ascontiguousarray(np.asarray(wx_w, np.float32)).astype(bf)

    def colbias(b):
        out = np.zeros((128, HT), np.float32)
        out[:] = np.asarray(b, np.float32).reshape(HT, 128).T
        return out

    colsel = np.zeros((128, FN * 128), np.float32)
    rowsel = np.zeros((128, FN * 128), np.float32)
    for f in range(FN):
        colsel[:, f * 128 + f] = 1.0
        rowsel[f, f * 128:(f + 1) * 128] = 1.0
    ones = np.ones((128, 128), np.float32)

    in_maps = []
    for r in range(NCORES):
        rows = slice(r * SH, (r + 1) * SH)
        # ptt[p, kt*SH + n] = S_P * W[r*SH + n, kt*128 + p]
        #                   = W8T[kt*128 + p, r*SH + n]
        # linearized [128, KT*SH]: contiguous per partition per kpair
        ptt = np.ascontiguousarray(
            W8T[:, rows].reshape(KT, 128, SH).transpose(1, 0, 2)
            .reshape(128, KT * SH)
        )
        xt = np.ascontiguousarray(xpad[rows].T).astype(bf)    # [IN, SH]
        in_maps.append(dict(
            xt=xt, ptt=ptt, w1=w1_b, w2=w2_b, wf=wf_bm, wx=wx_bm,
            b1=colbias(lin1_b), b2=colbias(lin2_b),
            wfb=colbias(wf_b), wxb=colbias(wx_b),
            alpha=alpha_bc, colsel=colsel.astype(bf),
            rowsel=rowsel.astype(bf), allones=ones.astype(bf),
        ))
    return in_maps


def run(in_maps, trace=False, **kw):
    nc = _get_graph()
    return run_bass_kernel_spmd(
        nc, in_maps, core_ids=list(range(NCORES)), trace=trace, **kw
    )


def kernel(**inputs):
    in_maps = prepare_in_maps(**inputs)
    res = run(in_maps)
    full = np.concatenate([res.results[r]["out"] for r in range(NCORES)], 0)
    return np.ascontiguousarray(full[:N, :N]).astype(np.float32)


# revision 7
# speedup vs baseline: 1.1255x; 1.1255x over previous
"""AMNet GNN message passing on 8 TRN2 NeuronCores.

Strategy (v3)
-------------
The graph propagation ``prop(v) = scatter_add(ew * v[src]) + 0.5*v`` is run as
5 dense GEMM hops against a host-materialized normalized operator, with the
(all-gathered) node features V as the *stationary* PE weights (fp8 DoubleRow)
and the resident P.T tiles streaming as the moving operand, producing hop
output feature-major ([HID, nodes_local]).

v3 changes vs v2 (trace-driven):
  - P.T (13.1MB/core) is host-linearized to [128, KT*SH] (contiguous per
    partition) and bulk-loaded as 40 per-kpair DMAs on the gpsimd SWDGE
    queue in KORDER. In v2 it sat at the head of the sync HWDGE queue and
    stalled the first exchange (and hence hop 1) until ~110us.
  - DMA queue split: exchange transposes + bounce writes + final output
    writes on the sync queue; reloads + xt/rhs loads on the scalar queue.
    Reloads are issued immediately after their AllGather (they can no
    longer head-of-line block the transposes).
  - Node segments rebalanced A=768 / B=512 (was 512/768): phase 1 is now
    the longer phase, giving the B AllGather ~17us of phase-1 contraction
    (KPA = 24 pairs) to land before the KPB tiles are consumed.
  - hf accumulation (hf_f += alpha * S_h) split per segment and spread
    across DVE and GpSimd so the next hop's PSUM drains don't queue
    behind it on the DVE FIFO.
  - Attention runs segment-major (full logits/softmax/res chain on
    segment A first) so the res-A AllGather fires ~25us earlier and the
    final distributed GEMM starts sooner.

Nodes are padded 10000 -> 10240; every core owns 1280 rows; SPMD on 8 cores.
"""

import sys

for _p in ("/opt/trn_rl_repo", "/root/.axon_site/_ro/trn_rl_repo"):
    if _p not in sys.path:
        sys.path.append(_p)

import math

import ml_dtypes
import numpy as np

import concourse.bass as bass
import concourse.mybir as mybir
import concourse.tile as tile
from concourse import bacc
from concourse.bass_utils import run_bass_kernel_spmd

N, E, IN, HID, KDEG, FN = 10000, 640000, 512, 256, 5, 5
NCORES = 8
NPAD = 10240                 # padded node count (8 * 1280)
SH = NPAD // NCORES          # 1280 nodes per core
MT = SH // 128               # 10 node tiles per core
KT = NPAD // 128             # 80 contraction tiles (all nodes)
KP = KT // 2                 # 40 DoubleRow contraction pairs
HT = HID // 128              # 2 feature tiles
INT = IN // 128              # 4 input-feature tiles

BF16 = mybir.dt.bfloat16
F32 = mybir.dt.float32
FP8 = mybir.dt.float8e4

S_P = 64.0                   # P fp8 scale (entries ~0.008 -> ~0.5)
S_V = 8.0                    # V fp8 scale
DESCALE = 2.0 / (S_P * S_V)  # hop drain: S_new = psum*DESCALE + S_old

# node segments (local): A = tiles 0..5 (768 nodes), B = tiles 6..9 (512)
NA, NB = 768, SH - 768
NTA, NTB = NA // 128, NB // 128          # 6 / 4 tiles
KP_R = MT // 2                           # 5 kpairs per rank
KPA_R, KPB_R = NTA // 2, NTB // 2        # 3 / 2 kpairs per rank per segment
# contraction k-pair order: pairs covering A-tiles of every rank first, so
# the next hop can start on AllGather-A data while segment B still exchanges
KPA = [r * KP_R + i for r in range(NCORES) for i in range(KPA_R)]
KPB = [r * KP_R + KPA_R + i for r in range(NCORES) for i in range(KPB_R)]
KORDER = KPA + KPB

# MLP strips: segment A = strips 0+1, segment B = strip 2
NSTRIP = [(0, 512), (512, 256), (768, 512)]
# per-segment strips for attention (psum <= 512 wide)
SEGSTRIP = {"a": [(0, 512), (512, 256)], "b": [(768, 512)]}
SEGOF = {"a": (0, NA), "b": (NA, NB)}


def _bern_coeff(K):
    out = np.zeros((K + 1, K + 1), dtype=np.float64)
    P = np.polynomial.polynomial.Polynomial
    for i in range(K + 1):
        c = np.zeros(i + 1)
        c[i] = math.comb(K, i)
        p = P(c) * (P([1.0, -1.0]) ** (K - i))
        out[i, : len(p.coef)] = p.coef
    return out


def _build_graph():
    nc = bacc.Bacc("TRN2", target_bir_lowering=False, debug=False, num_devices=NCORES)

    # ---- per-core parameters ----
    xt = nc.declare_dram_parameter("xt", [IN, SH], BF16, isOutput=False)
    ptt = nc.declare_dram_parameter("ptt", [128, KT * SH], FP8, isOutput=False)
    w1 = nc.declare_dram_parameter("w1", [IN, HID], BF16, isOutput=False)
    w2 = nc.declare_dram_parameter("w2", [HID, HID], BF16, isOutput=False)
    wf = nc.declare_dram_parameter("wf", [HID, HID], BF16, isOutput=False)
    wx = nc.declare_dram_parameter("wx", [HID, HID], BF16, isOutput=False)
    b1 = nc.declare_dram_parameter("b1", [128, HT], F32, isOutput=False)
    b2 = nc.declare_dram_parameter("b2", [128, HT], F32, isOutput=False)
    wfb = nc.declare_dram_parameter("wfb", [128, HT], F32, isOutput=False)
    wxb = nc.declare_dram_parameter("wxb", [128, HT], F32, isOutput=False)
    alpha = nc.declare_dram_parameter("alpha", [128, FN * (KDEG + 1)], F32,
                                      isOutput=False)
    colsel = nc.declare_dram_parameter("colsel", [128, FN * 128], BF16,
                                       isOutput=False)
    rowsel = nc.declare_dram_parameter("rowsel", [128, FN * 128], BF16,
                                       isOutput=False)
    allones = nc.declare_dram_parameter("allones", [128, 128], BF16,
                                        isOutput=False)
    out = nc.declare_dram_parameter("out", [SH, NPAD], BF16, isOutput=True)

    rg = [list(range(NCORES))]

    with tile.TileContext(nc) as tc:
        with tc.tile_pool(name="dram", bufs=1, space="DRAM") as dram, \
             tc.tile_pool(name="consts", bufs=1) as consts, \
             tc.tile_pool(name="persist", bufs=1) as persist:

            # ---- DRAM internals for collectives ----
            # hop exchanges e=0..4 (e=0 is the MLP output h); node-major fp8
            # v8 layout per core: [128, hh, nt*128] with node = t*128 + p
            bounce_a = [dram.tile([128, HT, NA], FP8, name=f"bnc_a{e}")
                        for e in range(KDEG)]
            bounce_b = [dram.tile([128, HT, NB], FP8, name=f"bnc_b{e}")
                        for e in range(KDEG)]
            vfull_a = [dram.tile([NCORES, 128, HT, NA], FP8, addr_space="Shared",
                                 name=f"vf_a{e}") for e in range(KDEG)]
            vfull_b = [dram.tile([NCORES, 128, HT, NB], FP8, addr_space="Shared",
                                 name=f"vf_b{e}") for e in range(KDEG)]
            # res exchange (bf16, feature-major: res.T[hid, n])
            bounce_ra = dram.tile([128, HT, NA], BF16)
            bounce_rb = dram.tile([128, HT, NB], BF16)
            vfull_ra = dram.tile([NCORES, 128, HT, NA], BF16, addr_space="Shared")
            vfull_rb = dram.tile([NCORES, 128, HT, NB], BF16, addr_space="Shared")
            # tiny warmup collective: absorbs the ~20-35us first-AllGather
            # firmware warmup during the MLP instead of on hop 0's exchange
            bounce_w = dram.tile([128, 4], BF16)
            vfull_w = dram.tile([NCORES, 128, 4], BF16, addr_space="Shared")

            # ---- constants to SBUF ----
            w1_sb = consts.tile([128, INT, HID], BF16)
            nc.sync.dma_start(w1_sb[:], w1[:, :].rearrange("(kt p) n -> p kt n", p=128))
            wu_sb = consts.tile([128, 4], BF16)
            nc.sync.dma_start(wu_sb[:], allones[:, 0:4])
            nc.sync.dma_start(bounce_w[:, :], wu_sb[:])
            nc.gpsimd.collective_compute(
                "AllGather", mybir.AluOpType.bypass,
                ins=[bounce_w.opt()], outs=[vfull_w.opt()], replica_groups=rg,
            )
            w2_sb = consts.tile([128, HT, HID], BF16)
            nc.sync.dma_start(w2_sb[:], w2[:, :].rearrange("(kt p) n -> p kt n", p=128))
            wf_sb = consts.tile([128, HT, HID], BF16)
            nc.sync.dma_start(wf_sb[:], wf[:, :].rearrange("(kt p) n -> p kt n", p=128))
            wx_sb = consts.tile([128, HT, HID], BF16)
            nc.sync.dma_start(wx_sb[:], wx[:, :].rearrange("(kt p) n -> p kt n", p=128))
            b1_sb = consts.tile([128, HT], F32)
            nc.sync.dma_start(b1_sb[:], b1[:, :])
            b2_sb = consts.tile([128, HT], F32)
            nc.sync.dma_start(b2_sb[:], b2[:, :])
            wfb_sb = consts.tile([128, HT], F32)
            nc.sync.dma_start(wfb_sb[:], wfb[:, :])
            wxb_sb = consts.tile([128, HT], F32)
            nc.sync.dma_start(wxb_sb[:], wxb[:, :])
            alpha_sb = consts.tile([128, FN * (KDEG + 1)], F32)
            nc.sync.dma_start(alpha_sb[:], alpha[:, :])
            colsel_sb = consts.tile([128, FN * 128], BF16)
            nc.sync.dma_start(colsel_sb[:], colsel[:, :])
            rowsel_sb = consts.tile([128, FN * 128], BF16)
            nc.sync.dma_start(rowsel_sb[:], rowsel[:, :])
            ones_sb = consts.tile([128, 128], BF16)
            nc.sync.dma_start(ones_sb[:], allones[:, :])

            # ---- persistent activations ----
            s_sb = persist.tile([128, HT, SH], BF16)      # S_h = 2^h B_h (feat-major)
            xp_sb = persist.tile([128, HT, SH], BF16)     # x_proj.T
            res_sb = persist.tile([128, HT, SH], BF16)    # res.T
            hf_sb = persist.tile([128, FN, HT, SH], BF16)  # h_filters.T accumulators

            def a_of(f, h):
                return alpha_sb[:, f * (KDEG + 1) + h: f * (KDEG + 1) + h + 1]

            with tc.tile_pool(name="hop", bufs=1) as hop, \
                 tc.tile_pool(name="hop_ps", bufs=1, space="PSUM") as hop_ps, \
                 tc.tile_pool(name="mlp_ps", bufs=2, space="PSUM") as mlp_ps:

                pt_sb = hop.tile([128, KT, SH], FP8)
                # resident P.T bulk load on the gpsimd SWDGE queue in KORDER:
                # contiguous 2*SH bytes per partition per kpair, and it never
                # blocks the latency-critical sync/scalar HWDGE queues
                for kp in KORDER:
                    nc.gpsimd.dma_start(
                        pt_sb[:, 2 * kp:2 * kp + 2, :],
                        ptt[:, 2 * kp * SH:(2 * kp + 2) * SH].rearrange(
                            "p (t n) -> p t n", n=SH),
                    )

                vt_ctx = tc.tile_pool(name="vt", bufs=1)
                vt_pool = vt_ctx.__enter__()

                # ---- exchange helpers ----
                def exchange(e, seg):
                    """Transpose S segment to node-major, cast fp8, AllGather."""
                    n0, nw = SEGOF[seg]
                    vt = vt_pool.tile([128, HT, SH], BF16, tag="vt",
                                      name=f"vt{e}{seg}")
                    v8 = vt_pool.tile([128, HT, SH], FP8, tag="v8",
                                      name=f"v8{e}{seg}")
                    bnc = (bounce_a[e] if seg == "a" else bounce_b[e])
                    vf = (vfull_a[e] if seg == "a" else vfull_b[e])
                    for hh in range(HT):
                        # [128 hid, nw nodes] -> [128 node-part, nw/128, 128 hid]
                        nc.sync.dma_start_transpose(
                            vt[:, hh, n0:n0 + nw].rearrange(
                                "p (t q) -> p t q", q=128),
                            s_sb[:, hh, n0:n0 + nw],
                        )
                        nc.scalar.activation(
                            v8[:, hh, n0:n0 + nw], vt[:, hh, n0:n0 + nw],
                            mybir.ActivationFunctionType.Copy, scale=S_V,
                        )
                        nc.sync.dma_start(bnc[:, hh, :], v8[:, hh, n0:n0 + nw])
                    nc.gpsimd.collective_compute(
                        "AllGather", mybir.AluOpType.bypass,
                        ins=[bnc.opt()], outs=[vf.opt()], replica_groups=rg,
                    )
                    return vf

                def reload(v_sb, vf, seg):
                    # scalar HWDGE queue: waits on the AllGather semaphore
                    # without blocking the sync-queue transposes/bounces
                    ntt = NTA if seg == "a" else NTB
                    for r in range(NCORES):
                        nc.scalar.dma_start(
                            v_sb[:, :, r * ntt:(r + 1) * ntt, :]
                            .rearrange("p h t q -> p h (t q)"),
                            vf[r],
                        )

                # ======== MLP: S_0 = h.T = (relu(x@W1+b1)@W2+b2).T ========
                # strip-ordered so the A-segment exchange fires ASAP
                with tc.tile_pool(name="mlp", bufs=1) as mlp:
                    xt_sb = mlp.tile([128, INT, SH], BF16)
                    xt_r = xt[:, :].rearrange("(kt p) n -> p kt n", p=128)
                    for kt in range(INT):
                        nc.scalar.dma_start(xt_sb[:, kt, :], xt_r[:, kt, :])

                    h1_sb = mlp.tile([128, HT, SH], BF16)
                    for si, (n0, nw) in enumerate(NSTRIP):
                        for hh in range(HT):
                            ps = mlp_ps.tile([128, 512], F32, name="mlp_ps")
                            for k in range(INT):
                                nc.tensor.matmul(
                                    ps[:, :nw],
                                    w1_sb[:, k, hh * 128:(hh + 1) * 128],
                                    xt_sb[:, k, n0:n0 + nw],
                                    start=(k == 0), stop=(k == INT - 1),
                                )
                            nc.scalar.activation(
                                h1_sb[:, hh, n0:n0 + nw], ps[:, :nw],
                                mybir.ActivationFunctionType.Relu,
                                bias=b1_sb[:, hh:hh + 1],
                            )
                        for hh in range(HT):
                            ps = mlp_ps.tile([128, 512], F32, name="mlp_ps2",
                                             tag="mlp_ps")
                            for k in range(HT):
                                nc.tensor.matmul(
                                    ps[:, :nw],
                                    w2_sb[:, k, hh * 128:(hh + 1) * 128],
                                    h1_sb[:, k, n0:n0 + nw],
                                    start=(k == 0), stop=(k == HT - 1),
                                )
                            nc.scalar.activation(
                                s_sb[:, hh, n0:n0 + nw], ps[:, :nw],
                                mybir.ActivationFunctionType.Identity,
                                bias=b2_sb[:, hh:hh + 1],
                            )
                        if si == 1:
                            vf0a = exchange(0, "a")
                    vf0b = exchange(0, "b")

                    # x_proj.T = tanh(wx.T @ h.T + wxb) - fills the AG wait
                    for hh in range(HT):
                        for n0, nw in NSTRIP:
                            ps = mlp_ps.tile([128, 512], F32, name="xp_ps",
                                             tag="mlp_ps")
                            for k in range(HT):
                                nc.tensor.matmul(
                                    ps[:, :nw],
                                    wx_sb[:, k, hh * 128:(hh + 1) * 128],
                                    s_sb[:, k, n0:n0 + nw],
                                    start=(k == 0), stop=(k == HT - 1),
                                )
                            nc.scalar.activation(
                                xp_sb[:, hh, n0:n0 + nw], ps[:, :nw],
                                mybir.ActivationFunctionType.Tanh,
                                bias=wxb_sb[:, hh:hh + 1],
                            )

                # hf init: hf_f = alpha'[f,0] * S_0
                for f in range(FN):
                    for hh in range(HT):
                        nc.scalar.activation(
                            hf_sb[:, f, hh, :], s_sb[:, hh, :],
                            mybir.ActivationFunctionType.Copy,
                            scale=a_of(f, 0),
                        )

                # v weights pool opens after the MLP pool frees its space
                # (split into A/B tiles so segment-B reloads don't serialize
                # the phase-1/2 matmuls that only read segment-A weights)
                vsb_ctx = tc.tile_pool(name="vsb", bufs=2)
                vsb_pool = vsb_ctx.__enter__()

                def valloc(h):
                    va = vsb_pool.tile([128, HT, NCORES * NTA, 128], FP8,
                                       tag="v_a", name=f"v_a{h}")
                    vb = vsb_pool.tile([128, HT, NCORES * NTB, 128], FP8,
                                       tag="v_b", name=f"v_b{h}")
                    return va, vb

                def vslice(va, vb, i, hh):
                    if i < len(KPA):
                        return va[:, hh, 2 * i:2 * i + 2, :]
                    j = i - len(KPA)
                    return vb[:, hh, 2 * j:2 * j + 2, :]

                v_cur = valloc(0)
                reload(v_cur[0], vf0a, "a")
                reload(v_cur[1], vf0b, "b")

                # hf updates per segment (issued while the other segment's
                # matmuls run) so the drains never queue behind a full-width
                # update burst; scalar-AP STT is DVE-only (Pool lacks the op)
                def hf_update(h, seg):
                    n0, nw = SEGOF[seg]
                    for f in range(FN):
                        eng = nc.vector
                        for hh in range(HT):
                            eng.scalar_tensor_tensor(
                                out=hf_sb[:, f, hh, n0:n0 + nw],
                                in0=s_sb[:, hh, n0:n0 + nw],
                                scalar=a_of(f, h),
                                in1=hf_sb[:, f, hh, n0:n0 + nw],
                                op0=mybir.AluOpType.mult,
                                op1=mybir.AluOpType.add,
                            )

                # ======== 5 propagation hops (feature-major output) ========
                # phase 1: segment A outputs (cols 0..767, psum 512+256)
                # phase 2: segment B outputs (cols 768..1279) accumulates into
                # psA1 AGAIN: its start=True write has a WAR dependency on the
                # phase-1 drain, which pins the drain/exchange-A chain right
                # after phase 1 regardless of how the scheduler orders the
                # matmul stream (a greedy scheduler otherwise back-fills the
                # reload-B stall with phase-2 work and delays exchange A)
                psA1 = [hop_ps.tile([128, 512], F32, name=f"psA1{hh}")
                        for hh in range(HT)]
                psA2 = [hop_ps.tile([128, 256], F32, name=f"psA2{hh}")
                        for hh in range(HT)]

                for h in range(1, KDEG + 1):
                    va, vb = v_cur
                    # phase 1: output nodes 0..767 (segment A)
                    for i, kp in enumerate(KORDER):
                        st, sp = (i == 0), (i == KP - 1)
                        for hh in range(HT):
                            nc.tensor.matmul(
                                psA1[hh][:, :],
                                vslice(va, vb, i, hh),
                                pt_sb[:, 2 * kp:2 * kp + 2, 0:512],
                                start=st, stop=sp,
                                perf_mode=mybir.MatmulPerfMode.DoubleRow,
                            )
                            nc.tensor.matmul(
                                psA2[hh][:, :],
                                vslice(va, vb, i, hh),
                                pt_sb[:, 2 * kp:2 * kp + 2, 512:NA],
                                start=st, stop=sp,
                                perf_mode=mybir.MatmulPerfMode.DoubleRow,
                            )
                    # drain A (fused off-diag descale + self-loop add, bf16)
                    for hh in range(HT):
                        nc.vector.scalar_tensor_tensor(
                            out=s_sb[:, hh, 0:512], in0=psA1[hh][:, :],
                            scalar=DESCALE, in1=s_sb[:, hh, 0:512],
                            op0=mybir.AluOpType.mult, op1=mybir.AluOpType.add,
                        )
                        nc.vector.scalar_tensor_tensor(
                            out=s_sb[:, hh, 512:NA], in0=psA2[hh][:, :],
                            scalar=DESCALE, in1=s_sb[:, hh, 512:NA],
                            op0=mybir.AluOpType.mult, op1=mybir.AluOpType.add,
                        )
                    if h < KDEG:
                        vfa = exchange(h, "a")
                        v_cur = valloc(h)
                        reload(v_cur[0], vfa, "a")
                    # phase 2: output nodes 768..1279 (segment B) -> psA1
                    for i, kp in enumerate(KORDER):
                        st, sp = (i == 0), (i == KP - 1)
                        for hh in range(HT):
                            nc.tensor.matmul(
                                psA1[hh][:, :],
                                vslice(va, vb, i, hh),
                                pt_sb[:, 2 * kp:2 * kp + 2, NA:SH],
                                start=st, stop=sp,
                                perf_mode=mybir.MatmulPerfMode.DoubleRow,
                            )
                    for hh in range(HT):
                        nc.vector.scalar_tensor_tensor(
                            out=s_sb[:, hh, NA:SH], in0=psA1[hh][:, :],
                            scalar=DESCALE, in1=s_sb[:, hh, NA:SH],
                            op0=mybir.AluOpType.mult, op1=mybir.AluOpType.add,
                        )
                    if h < KDEG:
                        vfb = exchange(h, "b")
                        reload(v_cur[1], vfb, "b")
                    hf_update(h, "a")
                    hf_update(h, "b")

                vsb_ctx.__exit__(None, None, None)
                vt_ctx.__exit__(None, None, None)

            # ======== attention fusion (feature-major, full-width chain) ====
            with tc.tile_pool(name="attn", bufs=1) as attn, \
                 tc.tile_pool(name="attn_ps", bufs=2, space="PSUM") as attn_ps, \
                 tc.tile_pool(name="sc_ps", bufs=3, space="PSUM") as sc_ps, \
                 tc.tile_pool(name="lg_ps", bufs=1, space="PSUM") as lg_ps, \
                 tc.tile_pool(name="tmp2", bufs=3) as tmp2:

                # hfp_f.T = tanh(wf.T @ hf_f.T + wfb)
                hfp_sb = attn.tile([128, FN, HT, SH], BF16)
                for f in range(FN):
                    for hh in range(HT):
                        for n0, nw in NSTRIP:
                            ps = attn_ps.tile([128, 512], F32, name="hfp_ps",
                                              tag="hfp_ps")
                            for k in range(HT):
                                nc.tensor.matmul(
                                    ps[:, :nw],
                                    wf_sb[:, k, hh * 128:(hh + 1) * 128],
                                    hf_sb[:, f, k, n0:n0 + nw],
                                    start=(k == 0), stop=(k == HT - 1),
                                )
                            nc.scalar.activation(
                                hfp_sb[:, f, hh, n0:n0 + nw], ps[:, :nw],
                                mybir.ActivationFunctionType.Tanh,
                                bias=wfb_sb[:, hh:hh + 1],
                            )

                # logits: logit[f, n] = sum_hid hfp_f.T * xp.T
                # accumulated across (f, hh) into one psum via column-select
                psL = [lg_ps.tile([128, nw], F32, name=f"psL{si}")
                       for si, (n0, nw) in enumerate(NSTRIP)]
                for f in range(FN):
                    for hh in range(HT):
                        tmp = tmp2.tile([128, SH], BF16, tag="lg_tmp")
                        nc.vector.tensor_mul(
                            out=tmp[:], in0=hfp_sb[:, f, hh, :],
                            in1=xp_sb[:, hh, :],
                        )
                        for si, (n0, nw) in enumerate(NSTRIP):
                            nc.tensor.matmul(
                                psL[si][:, :],
                                colsel_sb[:, f * 128:(f + 1) * 128],
                                tmp[:, n0:n0 + nw],
                                start=(f == 0 and hh == 0),
                                stop=(f == FN - 1 and hh == HT - 1),
                            )
                # exp (logits are tiny dot products; no max-sub needed)
                expT = attn.tile([FN, SH], BF16)
                for si, (n0, nw) in enumerate(NSTRIP):
                    nc.scalar.activation(
                        expT[:, n0:n0 + nw], psL[si][0:FN, :],
                        mybir.ActivationFunctionType.Exp,
                    )
                # broadcast sum over filters + reciprocal
                rinv = attn.tile([128, SH], F32)
                for si, (n0, nw) in enumerate(NSTRIP):
                    psS = sc_ps.tile([128, 512], F32, name="psS", tag="sc")
                    nc.tensor.matmul(
                        psS[:, :nw], ones_sb[0:FN, :], expT[:, n0:n0 + nw],
                        start=True, stop=True,
                    )
                    nc.vector.reciprocal(rinv[:, n0:n0 + nw], psS[:, :nw])
                # score_f broadcast to 128 partitions: (1 x exp_f) * rinv
                score_sb = attn.tile([128, FN, SH], BF16)
                for f in range(FN):
                    for si, (n0, nw) in enumerate(NSTRIP):
                        psb = sc_ps.tile([128, 512], F32, name="psb",
                                         tag="sc")
                        nc.tensor.matmul(
                            psb[:, :nw], rowsel_sb[0:FN, f * 128:(f + 1) * 128],
                            expT[0:FN, n0:n0 + nw],
                            start=True, stop=True,
                        )
                        nc.vector.tensor_mul(
                            out=score_sb[:, f, n0:n0 + nw], in0=psb[:, :nw],
                            in1=rinv[:, n0:n0 + nw],
                        )

                # res.T = sum_f score_f * hf_f.T (A segment first for the AG;
                # product terms split DVE/GpSimd, adds on DVE)
                for seg in ("a", "b"):
                    sn0, snw = SEGOF[seg]
                    for hh in range(HT):
                        nc.vector.tensor_mul(
                            out=res_sb[:, hh, sn0:sn0 + snw],
                            in0=score_sb[:, 0, sn0:sn0 + snw],
                            in1=hf_sb[:, 0, hh, sn0:sn0 + snw],
                        )
                        for f in range(1, FN):
                            eng = nc.gpsimd if f % 2 == 0 else nc.vector
                            tmp = tmp2.tile([128, HT, SH], BF16,
                                            tag=f"res_tmp{f % 2}")
                            eng.tensor_mul(
                                out=tmp[:, hh, sn0:sn0 + snw],
                                in0=score_sb[:, f, sn0:sn0 + snw],
                                in1=hf_sb[:, f, hh, sn0:sn0 + snw],
                            )
                            nc.vector.tensor_add(
                                out=res_sb[:, hh, sn0:sn0 + snw],
                                in0=res_sb[:, hh, sn0:sn0 + snw],
                                in1=tmp[:, hh, sn0:sn0 + snw],
                            )
                    bnc = bounce_ra if seg == "a" else bounce_rb
                    vf = vfull_ra if seg == "a" else vfull_rb
                    nc.sync.dma_start(bnc[:, :, :], res_sb[:, :, sn0:sn0 + snw])
                    nc.gpsimd.collective_compute(
                        "AllGather", mybir.AluOpType.bypass,
                        ins=[bnc.opt()], outs=[vf.opt()], replica_groups=rg,
                    )

            # ======== final distributed GEMM: out_r = res_r @ res.T (bf16) ====
            with tc.tile_pool(name="fin", bufs=1) as fin, \
                 tc.tile_pool(name="stage", bufs=3) as stage_pool, \
                 tc.tile_pool(name="fin_ps", bufs=6, space="PSUM") as fin_ps:
                rhs_sb = fin.tile([128, HT, NPAD], BF16)
                for r in range(NCORES):
                    nc.scalar.dma_start(
                        rhs_sb[:, :, r * SH:r * SH + NA], vfull_ra[r]
                    )
                for r in range(NCORES):
                    nc.scalar.dma_start(
                        rhs_sb[:, :, r * SH + NA:(r + 1) * SH], vfull_rb[r]
                    )
                out_r = out[:, :].rearrange("(t p) f -> p t f", p=128)
                CHUNKS_A = [(r * SH + c0, cw) for r in range(NCORES)
                            for c0, cw in ((0, 512), (512, 256))]
                CHUNKS_B = [(r * SH + NA, 512) for r in range(NCORES)]
                # all A-halves first: they only need the res-A AllGather, and
                # their output writes overlap the whole B sweep
                for half, chunks in (("a", CHUNKS_A), ("b", CHUNKS_B)):
                    for m in range(MT):
                        stg = stage_pool.tile([128, NPAD], BF16, tag="stage")
                        stg_r = stg.rearrange("p (r q) -> p r q", q=SH)
                        dst_r = out_r[:, m, :].rearrange("p (r q) -> p r q",
                                                         q=SH)
                        for ci, (c0, cw) in enumerate(chunks):
                            ps = fin_ps.tile([128, 512], F32, name="fin_ps")
                            for k in range(HT):
                                nc.tensor.matmul(
                                    ps[:, :cw],
                                    res_sb[:, k, m * 128:(m + 1) * 128],
                                    rhs_sb[:, k, c0:c0 + cw],
                                    start=(k == 0), stop=(k == HT - 1),
                                )
                            if ci % 2 == 1:
                                nc.scalar.copy(stg[:, c0:c0 + cw], ps[:, :cw])
                            else:
                                nc.vector.tensor_copy(stg[:, c0:c0 + cw],
                                                      ps[:, :cw])
                        if half == "a":
                            nc.sync.dma_start(dst_r[:, :, :NA],
                                              stg_r[:, :, :NA])
                        else:
                            nc.sync.dma_start(dst_r[:, :, NA:],
                                              stg_r[:, :, NA:])
    nc.compile()
    return nc


_GRAPH_CACHE = {}


def _get_graph():
    if "nc" not in _GRAPH_CACHE:
        _GRAPH_CACHE["nc"] = _build_graph()
    return _GRAPH_CACHE["nc"]


def prepare_in_maps(x, edge_index, lin1_w, lin1_b, lin2_w, lin2_b, filt_w,
                    wf_w, wf_b, wx_w, wx_b):
    x = np.asarray(x, np.float32)
    edge_index = np.asarray(edge_index)
    src = edge_index[0].astype(np.int64)
    dst = edge_index[1].astype(np.int64)

    # ---- host prep: dense normalized operator, OFF-DIAGONAL only, x S_P ----
    deg = np.zeros(N, np.float32)
    np.add.at(deg, src, np.float32(1.0))
    dinv = np.where(deg > 0, 1.0 / np.sqrt(deg), 0.0).astype(np.float32)
    ew = (-(dinv[src] * dinv[dst]) * (0.5 * S_P)).astype(np.float32)
    W = np.zeros((NPAD, NPAD), np.float32)
    np.add.at(W, (dst, src), ew)
    f8 = ml_dtypes.float8_e4m3
    W8 = W.astype(f8)
    del W
    W8T = np.ascontiguousarray(W8.T)
    del W8

    coeff = _bern_coeff(KDEG).astype(np.float32)
    fw = 1.0 / (1.0 + np.exp(-np.asarray(filt_w, np.float32)))
    al = (fw @ coeff).astype(np.float32)                  # [FN, KDEG+1]
    al = al * (0.5 ** np.arange(KDEG + 1))[None, :]       # absorb S_h = 2^h B_h
    alpha_bc = np.repeat(al.reshape(1, -1), 128, 0).astype(np.float32)

    xpad = np.zeros((NPAD, IN), np.float32)
    xpad[:N] = x

    bf = ml_dtypes.bfloat16
    w1_b = np.ascontiguousarray(np.asarray(lin1_w, np.float32)).astype(bf)
    w2_b = np.ascontiguousarray(np.asarray(lin2_w, np.float32)).astype(bf)
    wf_bm = np.ascontiguousarray(np.asarray(wf_w, np.float32)).astype(bf)
    wx_bm = np.ascontiguousarray(np.asarray(wx_w, np.float32)).astype(bf)

    def colbias(b):
        out = np.zeros((128, HT), np.float32)
        out[:] = np.asarray(b, np.float32).reshape(HT, 128).T
        return out

    colsel = np.zeros((128, FN * 128), np.float32)
    rowsel = np.zeros((128, FN * 128), np.float32)
    for f in range(FN):
        colsel[:, f * 128 + f] = 1.0
        rowsel[f, f * 128:(f + 1) * 128] = 1.0
    ones = np.ones((128, 128), np.float32)

    in_maps = []
    for r in range(NCORES):
        rows = slice(r * SH, (r + 1) * SH)
        # ptt[p, kt*SH + n] = S_P * W[r*SH + n, kt*128 + p]
        #                   = W8T[kt*128 + p, r*SH + n]
        # linearized [128, KT*SH]: contiguous per partition per kpair
        ptt = np.ascontiguousarray(
            W8T[:, rows].reshape(KT, 128, SH).transpose(1, 0, 2)
            .reshape(128, KT * SH)
        )
        xt = np.ascontiguousarray(xpad[rows].T).astype(bf)    # [IN, SH]
        in_maps.append(dict(
            xt=xt, ptt=ptt, w1=w1_b, w2=w2_b, wf=wf_bm, wx=wx_bm,
            b1=colbias(lin1_b), b2=colbias(lin2_b),
            wfb=colbias(wf_b), wxb=colbias(wx_b),
            alpha=alpha_bc, colsel=colsel.astype(bf),
            rowsel=rowsel.astype(bf), allones=ones.astype(bf),
        ))
    return in_maps


def run(in_maps, trace=False, **kw):
    nc = _get_graph()
    return run_bass_kernel_spmd(
        nc, in_maps, core_ids=list(range(NCORES)), trace=trace, **kw
    )


def kernel(**inputs):
    in_maps = prepare_in_maps(**inputs)
    res = run(in_maps)
    full = np.concatenate([res.results[r]["out"] for r in range(NCORES)], 0)
    return np.ascontiguousarray(full[:N, :N]).astype(np.float32)
